# revision 13
# baseline (speedup 1.0000x reference)
"""CRF loss (multi-annotator) Trainium2 kernel — v2.

Problem (hardcoded): scores (8,200,64,32,32) f32, targets (8,200,64) int,
mask (200,64) bool, a_mask (8,64) bool -> scalar f32 loss.

Sharding: one annotator per NeuronCore (8 cores). Host applies a_mask and
sums / B.

Design (v2):
  - Sequence split into two independent serial chains: FORWARD over steps
    1..99 (mask-free: all lens >= 100) and BACKWARD over steps 199..100.
    log_Z[b] = ln <p_fwd99[b], w_bwd100[b]> + C*(sb+1), where the backward
    chain computes w_k = M_k ... M_sb 1_END via injection of E_k[:,END] at
    each batch's cutoff step (per-partition s0 kill + inject-matmuls with
    host-masked lhsT).  Two chains halve the serial length and hide the
    scan->mm->scan latency.
  - Layout rows=(half2, b64): all 64 batch elements on partitions. One
    [128,512] fused DVE scan (MUL_CUMSUM_SCALE) per direction per step;
    16-wide segment sums drop out as matmul-pairs (+lhsT @ hi-samples
    - lhsT @ lo-samples accumulated in PSUM, fp32 rhs straight from the
    cumsum tile) -> no tensor_sub, no PSUM->SBUF hops; the state is read
    by the next scan directly from PSUM.
  - No renorm anywhere: exp bias -ln(52.76) folded into the ACT exp
    (free); drift over 100 steps stays well inside f32. One Ln at the join.
  - exp on ACT in [128,2048] blocks (bias fused), double-buffered with the
    DMA stream; fwd stream carries steps 0..99, bwd stream steps 199..100,
    so each score element is streamed exactly once (26 MB bf16 per core).
  - tg energy: dma_gather of 256B blocks from an s-major bf16 copy
    (unchanged from v1), masked-sum STTs late in the scan, dup matmul.
"""

import os
import sys

import numpy as np

if os.path.isdir("/opt/trn_rl_repo"):
    sys.path.insert(0, "/opt/trn_rl_repo")

import ml_dtypes  # noqa: E402

import concourse.bass as bass  # noqa: E402
import concourse.tile as tile  # noqa: E402
from concourse import bacc, mybir  # noqa: E402
from concourse.bass_utils import run_bass_kernel_spmd  # noqa: E402

F32 = mybir.dt.float32
BF16 = mybir.dt.bfloat16
I16 = mybir.dt.int16

A, S, B, T = 8, 200, 64, 32
START_TAG, END_TAG = 30, 31
SF = 100        # fwd: steps 0..SF-1 (scan 1..SF-1); bwd: steps S-1..SF
NW = SF - 1     # wall steps in the main loop (99)
SBLK = 4        # steps per streamed DMA block
NBLK = SF // SBLK  # 25 blocks per direction
GBLK = 16       # steps per dma_gather chunk
CEXP = 3.9656   # exp bias: E = exp(x - CEXP)

# ---------------------------------------------------------------------------
# Custom DVE op: out[k] = running_sum(in0*in1*s0) (inclusive, whole stream)
# ---------------------------------------------------------------------------


def _register_mul_segscan():
    """out[p, s, k] = cumsum_k(in0[p,s,k] * in1[p,s,k] * s0[p]) with the
    running sum RESET at each page boundary (segmented scan over the
    innermost dim of a [P, S, N] access pattern).

    Built from the stock lower() pieces: same placement as the plain
    mul-cumsum scan, plus the SUB_DIM_DONE -> step-state transition (the
    proven PageIdx FSM shape) whose single-element override re-seeds the
    accumulator with the incoming product (BYPASS instead of ADD).
    """
    import dataclasses

    import concourse.dve_ops as dve_ops
    from concourse.dve_ops import OPS, DveOp, DveOpSpec
    from concourse import dve_spec as ds
    from concourse.dve_spec import AluOp, Spec, Src0, Src1, C0, scan
    from concourse.dve_uop import AluInp, Trigger

    name = "MUL_SEGSCAN_SCALE"
    for op in OPS:
        if op.name == name:
            return op

    def _ref(in0, in1, s0):
        prod = in0.astype(np.float32) * in1 * s0
        return np.cumsum(prod, axis=-1)

    spec = Spec(body=scan(AluOp.ADD, Src0 * Src1 * C0), reference=_ref)

    def lower_seg(spec, ver):
        from concourse.dve_uop import N_LANES, N_STAGES

        n_lanes, n_stages = N_LANES[ver], N_STAGES[ver]
        ds._validate_body(spec, ver)
        spec2 = ds._hoist_stream_invariant_ops(spec)
        scans = ds._collect(spec2.body, ds.Scan)
        latches = ds._collect(spec2.body, ds.Latch)
        placement = ds._build_placement(spec2, scans, n_stages, n_lanes)
        states = ds._build_state_machine(spec2, scans, latches, placement)
        assert len(states) == 2, states  # [seed, steady]
        steady = states[1]
        # steady: also fire step at each sub-dim boundary
        states[1] = dataclasses.replace(
            steady,
            trigger=(Trigger.SRC_TENSOR_DONE, Trigger.SUB_DIM_DONE, Trigger.NONE),
            next=(0, 2, 0),
        )
        # step: one element with the scan stage re-seeded from the product
        (sc_node,) = scans
        d = placement.node_stage[sc_node]
        states.append(
            dataclasses.replace(
                steady,
                overrides={d: ds._Stage(AluOp.BYPASS, AluInp.PREV_ALU_OUT)},
                trigger=(Trigger.SRC_TENSOR_DONE, Trigger.SUB_DIM_DONE, Trigger.COUNT),
                next=(0, 2, 1),
                repeat=1,
            )
        )
        out = [ds._assemble(st) for st in states]
        for u in out:
            u.validate(ver)
        return out

    row = dve_ops._CUSTOM_DVE_ROW_BASE + len(OPS)
    shas = {}
    for ver in ("v3", "v4"):
        shas[ver] = DveOpSpec(
            name=name, opcode=row, uops=lower_seg(spec, ver), rd1_en=True
        ).sha(ver)
    op = DveOp(name, spec, subdim=True, uops_sha=shas)
    # DveOp.__post_init__/compile paths re-lower via the stock lower();
    # patch this op's compiled entry into the cache so our uops are used.
    for ver in ("v3", "v4"):
        dve_ops._COMPILE_CACHE[(name, ver)] = DveOpSpec(
            name=name, opcode=row, uops=lower_seg(spec, ver), rd1_en=True
        )
    OPS.append(op)
    dve_ops.CUSTOM_DVE_SPECS[name] = spec
    dve_ops._SUB_OPCODE_FOR_NAME[name] = row
    return op


MUL_SEGSCAN_SCALE = _register_mul_segscan()


def _plan(S):
    """Gather chunk plan: list of (s0, nsteps, idx_col0, out_col0)."""
    chunks = []
    s0 = 0
    idx_col = 0
    out_col = 0
    while s0 < S:
        ns = min(GBLK, S - s0)
        ni = ns * B
        assert ni % 128 == 0
        chunks.append((s0, ns, idx_col, out_col))
        idx_col += ni // 16
        out_col += ni // 128
        s0 += ns
    return chunks, idx_col, out_col


def build_nc(hitsteps: frozenset):
    from contextlib import ExitStack

    chunks, idx_cols, out_blocks = _plan(S)
    NCH2 = 2 * len(chunks)

    nc = bacc.Bacc("TRN2", target_bir_lowering=False, debug=False, num_devices=8)

    ef_d = nc.dram_tensor("ef", [128, SF * 512], BF16, kind="ExternalInput").ap()
    eb_d = nc.dram_tensor("eb", [128, SF * 512], BF16, kind="ExternalInput").ap()
    tgv_d = nc.dram_tensor("tgv", [64, 256], F32, kind="ExternalInput").ap()
    mkf_d = nc.dram_tensor("mkf", [64, 256], F32, kind="ExternalInput").ap()
    s0b_d = nc.dram_tensor("s0b", [128, SF], F32, kind="ExternalInput").ap()
    injt_d = nc.dram_tensor("injt", [128, SF * 64], BF16, kind="ExternalInput").ap()
    m99_d = nc.dram_tensor("m99", [128, 16], F32, kind="ExternalInput").ap()
    end99_d = nc.dram_tensor("end99", [128, 16], F32, kind="ExternalInput").ap()
    cs_d = nc.dram_tensor("cs", [64, 1], F32, kind="ExternalInput").ap()
    lhsA_d = nc.dram_tensor("lhsA", [128, 64], BF16, kind="ExternalInput").ap()
    lhsI_d = nc.dram_tensor("lhsI", [128, 64], BF16, kind="ExternalInput").ap()
    lhsJ_d = nc.dram_tensor("lhsJ", [128, 64], F32, kind="ExternalInput").ap()
    out_d = nc.dram_tensor("losses", [64, 1], F32, kind="ExternalOutput").ap()

    with tile.TileContext(nc) as tc, ExitStack() as ctx:
        state = ctx.enter_context(tc.tile_pool(name="state", bufs=1))
        blkp = {
            d: ctx.enter_context(tc.tile_pool(name=f"blk{d}", bufs=3))
            for d in ("f", "b")
        }
        e16p = {
            d: ctx.enter_context(tc.tile_pool(name=f"e16{d}", bufs=3))
            for d in ("f", "b")
        }
        work = ctx.enter_context(tc.tile_pool(name="work", bufs=4))
        psf = ctx.enter_context(tc.tile_pool(name="psf", bufs=2, space="PSUM"))
        psb = ctx.enter_context(tc.tile_pool(name="psb", bufs=2, space="PSUM"))
        psumg = ctx.enter_context(tc.tile_pool(name="psumg", bufs=2, space="PSUM"))

        # ---- persistent tiles ----
        ones = state.tile([128, 1], F32)
        biast = state.tile([128, 1], F32)
        s0bt = state.tile([128, SF], F32)
        injt = state.tile([128, SF * 64], BF16)
        m99t = state.tile([128, 16], F32)
        end99t = state.tile([128, 16], F32)
        cst = state.tile([64, 1], F32)
        lhsA = state.tile([128, 64], BF16)
        lhsI = state.tile([128, 64], BF16)
        lhsJ = state.tile([128, 64], F32)
        scf = [state.tile([128, 512], BF16, name=f"scf{i}") for i in range(2)]
        scb = [state.tile([128, 512], BF16, name=f"scb{i}") for i in range(2)]
        tgv = state.tile([64, 256], F32)
        mkf = state.tile([64, 256], F32)

        # critical-path tables first (tiny), then the first data blocks in
        # halves (exp starts earlier); join-only tables are loaded late.
        nc.vector.memset(ones[:], 1.0)
        nc.vector.memset(biast[:], -CEXP)

        nc.sync.dma_start(lhsI[:], lhsI_d[:])
        nc.sync.dma_start(lhsA[:], lhsA_d[:])
        nc.sync.dma_start(injt[:, 0:64], injt_d[:, 0:64])
        nc.sync.dma_start(s0bt[:], s0b_d[:])

        # ---- streamed blocks + exp ----
        def load_block(d, bi, halves=1):
            src = ef_d if d == "f" else eb_d
            blk = blkp[d].tile([128, SBLK * 512], BF16, tag="blk", name=f"blkt{d}")
            c0 = bi * 2048
            hw_ = 2048 // halves
            for h in range(halves):
                nc.sync.dma_start(
                    blk[:, h * hw_ : (h + 1) * hw_],
                    src[:, c0 + h * hw_ : c0 + (h + 1) * hw_],
                )
            return blk

        def exp_block(d, blk, halves=1):
            e16 = e16p[d].tile([128, SBLK * 512], BF16, tag="e16", name=f"e16t{d}")
            hw_ = 2048 // halves
            for h in range(halves):
                nc.scalar.activation(
                    e16[:, h * hw_ : (h + 1) * hw_],
                    blk[:, h * hw_ : (h + 1) * hw_],
                    mybir.ActivationFunctionType.Exp,
                    bias=biast[:],
                )
            return e16

        blk = {d: load_block(d, 0, halves=4) for d in ("f", "b")}
        e16 = {d: exp_block(d, blk[d], halves=4) for d in ("f", "b")}
        blk_next = {d: load_block(d, 1) for d in ("f", "b")}
        e16_next = {d: exp_block(d, blk_next[d]) for d in ("f", "b")}
        blk_next2 = {d: load_block(d, 2) for d in ("f", "b")}
        e16_next2 = {d: exp_block(d, blk_next2[d]) for d in ("f", "b")}

        # join-only tables: needed only at the tail
        nc.sync.dma_start(injt[:, 64:], injt_d[:, 64:])
        nc.sync.dma_start(m99t[:], m99_d[:])
        nc.sync.dma_start(end99t[:], end99_d[:])
        nc.sync.dma_start(cst[:], cs_d[:])
        nc.sync.dma_start(lhsJ[:], lhsJ_d[:])
        nc.sync.dma_start(tgv[:], tgv_d[:])
        nc.sync.dma_start(mkf[:], mkf_d[:])

        # ---- fwd init: state[(h,b), j] = E_0[b, START=(1,14), (h,j)] ----
        ptf = psf.tile([128, 16], F32, tag="ptf")
        nc.tensor.matmul(
            ptf[0:64, :], lhsI[:], e16["f"][:, 14:256:16], start=True, stop=True
        )
        nc.tensor.matmul(
            ptf[64:128, :], lhsI[:], e16["f"][:, 270:512:16], start=True, stop=True
        )

        # ---- bwd init (k=199, jidx=0): inject-only ----
        ptb = psb.tile([128, 16], F32, tag="ptb")
        nc.tensor.matmul(
            ptb[0:64, :], injt[:, 0:64], e16["b"][:, 15:256:16], start=True, stop=True
        )
        nc.tensor.matmul(
            ptb[64:128, :], injt[:, 0:64], e16["b"][:, 271:512:16],
            start=True, stop=True,
        )

        # PE-warming: dependency-free dummy matmul per wall step keeps the
        # HAM duty cycle high so the real (tiny) matmuls run at 2.4 GHz.
        warm = psumg.tile([64, 512], F32, tag="warm")

        # ---- main loop: wall step w handles fwd step 1+w, bwd jidx 1+w ----
        for w in range(NW):
            j = 1 + w                 # fwd step index == bwd stream index
            bi, sl = divmod(j, SBLK)
            if sl == 0:
                for d in ("f", "b"):
                    blk[d] = blk_next[d]
                    e16[d] = e16_next[d]
                blk_next = blk_next2
                e16_next = e16_next2
                if bi + 2 < NBLK:
                    blk_next2 = {d: load_block(d, bi + 2) for d in ("f", "b")}
                    e16_next2 = {d: exp_block(d, blk_next2[d]) for d in ("f", "b")}

            # fwd: scan -> 4 seg mms
            sc = scf[w % 2]
            nc.vector._custom_dve(
                MUL_SEGSCAN_SCALE,
                out=sc[:].rearrange("p (s n) -> p s n", n=16),
                in0=e16["f"][:, sl * 512 : (sl + 1) * 512].rearrange(
                    "p (s n) -> p s n", n=16
                ),
                in1=ptf[:].unsqueeze(1).broadcast_to([128, 32, 16]),
                s0=ones[:],
            )
            ptf = psf.tile([128, 16], F32, tag="ptf")
            nc.tensor.matmul(ptf[0:64, :], lhsA[:], sc[:, 15:256:16], start=True, stop=True)
            nc.tensor.matmul(ptf[64:128, :], lhsA[:], sc[:, 271:512:16], start=True, stop=True)

            # bwd: scan (s0 kill) -> 4 seg mms + 2 inject mms
            sb_ = scb[w % 2]
            nc.vector._custom_dve(
                MUL_SEGSCAN_SCALE,
                out=sb_[:].rearrange("p (s n) -> p s n", n=16),
                in0=e16["b"][:, sl * 512 : (sl + 1) * 512].rearrange(
                    "p (s n) -> p s n", n=16
                ),
                in1=ptb[:].unsqueeze(1).broadcast_to([128, 32, 16]),
                s0=s0bt[:, j : j + 1],
            )
            ptb = psb.tile([128, 16], F32, tag="ptb")
            hit = j in hitsteps
            nc.tensor.matmul(
                ptb[0:64, :], lhsA[:], sb_[:, 15:256:16], start=True, stop=not hit
            )
            nc.tensor.matmul(
                ptb[64:128, :], lhsA[:], sb_[:, 271:512:16], start=True, stop=not hit
            )
            if hit:
                nc.tensor.matmul(
                    ptb[0:64, :], injt[:, j * 64 : j * 64 + 64],
                    e16["b"][:, sl * 512 + 15 : sl * 512 + 256 : 16],
                    start=False, stop=True,
                )
                nc.tensor.matmul(
                    ptb[64:128, :], injt[:, j * 64 : j * 64 + 64],
                    e16["b"][:, sl * 512 + 271 : sl * 512 + 512 : 16],
                    start=False, stop=True,
                )

            nc.tensor.matmul(
                warm[:], lhsA[:], injt[:, 0:512], start=True, stop=True,
            )

        # ---- tg energy: masked sum of host-extracted target scores ----
        tgE = state.tile([64, 1], F32)
        tgtmp = work.tile([64, 256], F32, tag="tgtmp")
        nc.vector.scalar_tensor_tensor(
            tgtmp[:],
            tgv[:],
            1.0,
            mkf[:],
            op0=mybir.AluOpType.mult,
            op1=mybir.AluOpType.mult,
            accum_out=tgE[:],
        )

        # ---- join ----
        w2 = state.tile([128, 16], F32)
        nc.vector.tensor_mul(w2[:], ptb[:], m99t[:])
        nc.vector.tensor_add(w2[:], w2[:], end99t[:])
        prod = state.tile([128, 16], F32)
        nc.vector.tensor_mul(prod[:], w2[:], ptf[:])
        dsum = state.tile([128, 1], F32)
        nc.vector.reduce_sum(dsum[:], prod[:], axis=mybir.AxisListType.X)
        dps = psumg.tile([64, 1], F32, tag="d")
        nc.tensor.matmul(dps[:], lhsJ[:], dsum[:], start=True, stop=True)
        lnz = state.tile([64, 1], F32)
        nc.scalar.activation(lnz[:], dps[:], mybir.ActivationFunctionType.Ln)
        res = state.tile([64, 1], F32)
        nc.vector.tensor_add(res[:], lnz[:], cst[:])
        nc.vector.tensor_sub(res[:], res[:], tgE[:])
        nc.sync.dma_start(out_d[:], res[:])

    nc.compile()
    return nc


def host_prep(scores_a: np.ndarray, targets_a: np.ndarray, mask: np.ndarray):
    """Per-annotator tensors for the v2 kernel."""
    chunks, idx_cols, out_blocks = _plan(S)

    lens = mask.astype(np.int64).sum(axis=0)  # (B,)
    assert lens.min() >= S // 2, "kernel assumes valid-prefix lens >= S//2"
    sbv = lens - 1  # cutoff step per b in [99, 199]

    x = scores_a.reshape(S, B, 2, 16, 2, 16)  # s b h j th tl
    arr_f = np.ascontiguousarray(
        x[:SF].transpose(2, 1, 0, 4, 5, 3)       # h b s th tl j
    ).astype(ml_dtypes.bfloat16).reshape(128, SF * 512)
    # bwd: rows (tt,b); col (jidx, hf, fl, tl); jidx -> k = 199 - jidx
    xb = x[SF:][::-1]                             # jidx b hf fl tt tl
    arr_b = np.ascontiguousarray(
        xb.transpose(4, 1, 0, 2, 3, 5)            # tt b jidx hf fl tl
    ).astype(ml_dtypes.bfloat16).reshape(128, SF * 512)

    # s0 kill + inject tables (rows (x2, b64))
    r = np.arange(128)
    br = r % 64
    s0b = np.ones((128, SF), dtype=np.float32)
    injt = np.zeros((128, SF, 64), dtype=np.float32)
    lhsI_base = ((br[:, None] == np.arange(64)[None, :]) & (r[:, None] >= 64))
    for jidx in range(SF):
        k = S - 1 - jidx
        hit = sbv == k                            # (B,)
        s0b[:, jidx] = (~hit)[br]
        injt[:, jidx, :] = lhsI_base * hit[None, :]
    injt = injt.reshape(128, SF * 64).astype(ml_dtypes.bfloat16)

    m99 = np.repeat((~(sbv == SF - 1))[br].astype(np.float32)[:, None], 16, axis=1)
    end99 = np.zeros((128, 16), dtype=np.float32)
    for b in range(B):
        if sbv[b] == SF - 1:
            end99[64 + b, 15] = 1.0
    cs = (CEXP * (sbv + 1)).astype(np.float32)[:, None]

    lhsAf = (br[:, None] == np.arange(64)[None, :]).astype(np.float32)
    lhsA = lhsAf.astype(ml_dtypes.bfloat16)
    lhsI = lhsI_base.astype(ml_dtypes.bfloat16)
    lhsJ = lhsAf.copy()

    # tg values: host-side indexed extraction (pure data movement);
    # the mask multiply + sum stay on device.
    tgt = targets_a.astype(np.int64)              # (S, B)
    flat = scores_a.reshape(S, B, T * T)
    tgvals = np.take_along_axis(flat, tgt[..., None], axis=2)[..., 0]  # (S, B)
    tgv = np.zeros((64, 256), dtype=np.float32)
    tgv[:, :S] = tgvals.T
    mkf = np.zeros((64, 256), dtype=np.float32)
    mkf[:, :S] = mask.T.astype(np.float32)

    return dict(
        ef=arr_f, eb=arr_b, tgv=tgv, mkf=mkf,
        s0b=s0b, injt=injt, m99=m99, end99=end99, cs=cs,
        lhsA=lhsA, lhsI=lhsI, lhsJ=lhsJ,
    )


_NC_CACHE = {}

TRACE = False
TRACE_DIR = None
LAST_RESULTS = None


def _get_nc(hitsteps: frozenset):
    if hitsteps not in _NC_CACHE:
        _NC_CACHE[hitsteps] = build_nc(hitsteps)
    return _NC_CACHE[hitsteps]


def kernel(scores, targets, mask, a_mask):
    scores = np.asarray(scores)
    targets = np.asarray(targets)
    mask_np = np.asarray(mask).astype(bool)
    a_mask_np = np.asarray(a_mask).astype(bool)

    lens = mask_np.astype(np.int64).sum(axis=0)
    sbv = lens - 1
    hitsteps = frozenset(int(S - 1 - k) for k in sbv if k >= SF) - {0}
    nc = _get_nc(hitsteps)

    in_maps = []
    for a in range(A):
        in_maps.append(host_prep(scores[a], targets[a], mask_np))

    if TRACE:
        import antenv

        shim = "/opt/trn_rl_repo/antenv"
        if os.path.isdir(shim) and shim not in list(antenv.__path__):
            antenv.__path__.append(shim)

    global LAST_RESULTS
    res = run_bass_kernel_spmd(
        nc, in_maps, core_ids=list(range(A)), trace=TRACE, tmpdir=TRACE_DIR
    )
    LAST_RESULTS = res
    losses = np.stack([r["losses"][:, 0] for r in res.results])  # (A, B)
    loss = np.where(a_mask_np, losses, 0.0).sum(dtype=np.float32) / np.float32(B)
    return np.float32(loss)


# revision 15
# speedup vs baseline: 1.0126x; 1.0126x over previous
"""CRF loss (multi-annotator) Trainium2 kernel — v2.

Problem (hardcoded): scores (8,200,64,32,32) f32, targets (8,200,64) int,
mask (200,64) bool, a_mask (8,64) bool -> scalar f32 loss.

Sharding: one annotator per NeuronCore (8 cores). Host applies a_mask and
sums / B.

Design (v2):
  - Sequence split into two independent serial chains: FORWARD over steps
    1..99 (mask-free: all lens >= 100) and BACKWARD over steps 199..100.
    log_Z[b] = ln <p_fwd99[b], w_bwd100[b]> + C*(sb+1), where the backward
    chain computes w_k = M_k ... M_sb 1_END via injection of E_k[:,END] at
    each batch's cutoff step (per-partition s0 kill + inject-matmuls with
    host-masked lhsT).  Two chains halve the serial length and hide the
    scan->mm->scan latency.
  - Layout rows=(half2, b64): all 64 batch elements on partitions. One
    [128,512] fused DVE scan (MUL_CUMSUM_SCALE) per direction per step;
    16-wide segment sums drop out as matmul-pairs (+lhsT @ hi-samples
    - lhsT @ lo-samples accumulated in PSUM, fp32 rhs straight from the
    cumsum tile) -> no tensor_sub, no PSUM->SBUF hops; the state is read
    by the next scan directly from PSUM.
  - No renorm anywhere: exp bias -ln(52.76) folded into the ACT exp
    (free); drift over 100 steps stays well inside f32. One Ln at the join.
  - exp on ACT in [128,2048] blocks (bias fused), double-buffered with the
    DMA stream; fwd stream carries steps 0..99, bwd stream steps 199..100,
    so each score element is streamed exactly once (26 MB bf16 per core).
  - tg energy: dma_gather of 256B blocks from an s-major bf16 copy
    (unchanged from v1), masked-sum STTs late in the scan, dup matmul.
"""

import os
import sys

import numpy as np

if os.path.isdir("/opt/trn_rl_repo"):
    sys.path.insert(0, "/opt/trn_rl_repo")

import ml_dtypes  # noqa: E402

import concourse.bass as bass  # noqa: E402
import concourse.tile as tile  # noqa: E402
from concourse import bacc, mybir  # noqa: E402
from concourse.bass_utils import run_bass_kernel_spmd  # noqa: E402

F32 = mybir.dt.float32
BF16 = mybir.dt.bfloat16
I16 = mybir.dt.int16

A, S, B, T = 8, 200, 64, 32
START_TAG, END_TAG = 30, 31
SF = 100        # fwd: steps 0..SF-1 (scan 1..SF-1); bwd: steps S-1..SF
NW = SF - 1     # wall steps in the main loop (99)
SBLK = 4        # steps per streamed DMA block
NBLK = SF // SBLK  # 25 blocks per direction
GBLK = 16       # steps per dma_gather chunk
CEXP = 3.9656   # exp bias: E = exp(x - CEXP)

# ---------------------------------------------------------------------------
# Custom DVE op: out[k] = running_sum(in0*in1*s0) (inclusive, whole stream)
# ---------------------------------------------------------------------------


def _register_mul_segscan():
    """out[p, s, k] = cumsum_k(in0[p,s,k] * in1[p,s,k] * s0[p]) with the
    running sum RESET at each page boundary (segmented scan over the
    innermost dim of a [P, S, N] access pattern).

    Built from the stock lower() pieces: same placement as the plain
    mul-cumsum scan, plus the SUB_DIM_DONE -> step-state transition (the
    proven PageIdx FSM shape) whose single-element override re-seeds the
    accumulator with the incoming product (BYPASS instead of ADD).
    """
    import dataclasses

    import concourse.dve_ops as dve_ops
    from concourse.dve_ops import OPS, DveOp, DveOpSpec
    from concourse import dve_spec as ds
    from concourse.dve_spec import AluOp, Spec, Src0, Src1, C0, scan
    from concourse.dve_uop import AluInp, Trigger

    name = "MUL_SEGSCAN_SCALE"
    for op in OPS:
        if op.name == name:
            return op

    def _ref(in0, in1, s0):
        prod = in0.astype(np.float32) * in1 * s0
        return np.cumsum(prod, axis=-1)

    spec = Spec(body=scan(AluOp.ADD, Src0 * Src1 * C0), reference=_ref)

    def lower_seg(spec, ver):
        from concourse.dve_uop import N_LANES, N_STAGES

        n_lanes, n_stages = N_LANES[ver], N_STAGES[ver]
        ds._validate_body(spec, ver)
        spec2 = ds._hoist_stream_invariant_ops(spec)
        scans = ds._collect(spec2.body, ds.Scan)
        latches = ds._collect(spec2.body, ds.Latch)
        placement = ds._build_placement(spec2, scans, n_stages, n_lanes)
        states = ds._build_state_machine(spec2, scans, latches, placement)
        assert len(states) == 2, states  # [seed, steady]
        steady = states[1]
        # steady: also fire step at each sub-dim boundary
        states[1] = dataclasses.replace(
            steady,
            trigger=(Trigger.SRC_TENSOR_DONE, Trigger.SUB_DIM_DONE, Trigger.NONE),
            next=(0, 2, 0),
        )
        # step: one element with the scan stage re-seeded from the product
        (sc_node,) = scans
        d = placement.node_stage[sc_node]
        states.append(
            dataclasses.replace(
                steady,
                overrides={d: ds._Stage(AluOp.BYPASS, AluInp.PREV_ALU_OUT)},
                trigger=(Trigger.SRC_TENSOR_DONE, Trigger.SUB_DIM_DONE, Trigger.COUNT),
                next=(0, 2, 1),
                repeat=1,
            )
        )
        out = [ds._assemble(st) for st in states]
        for u in out:
            u.validate(ver)
        return out

    row = dve_ops._CUSTOM_DVE_ROW_BASE + len(OPS)
    shas = {}
    for ver in ("v3", "v4"):
        shas[ver] = DveOpSpec(
            name=name, opcode=row, uops=lower_seg(spec, ver), rd1_en=True
        ).sha(ver)
    op = DveOp(name, spec, subdim=True, uops_sha=shas)
    # DveOp.__post_init__/compile paths re-lower via the stock lower();
    # patch this op's compiled entry into the cache so our uops are used.
    for ver in ("v3", "v4"):
        dve_ops._COMPILE_CACHE[(name, ver)] = DveOpSpec(
            name=name, opcode=row, uops=lower_seg(spec, ver), rd1_en=True
        )
    OPS.append(op)
    dve_ops.CUSTOM_DVE_SPECS[name] = spec
    dve_ops._SUB_OPCODE_FOR_NAME[name] = row
    return op


MUL_SEGSCAN_SCALE = _register_mul_segscan()


def _plan(S):
    """Gather chunk plan: list of (s0, nsteps, idx_col0, out_col0)."""
    chunks = []
    s0 = 0
    idx_col = 0
    out_col = 0
    while s0 < S:
        ns = min(GBLK, S - s0)
        ni = ns * B
        assert ni % 128 == 0
        chunks.append((s0, ns, idx_col, out_col))
        idx_col += ni // 16
        out_col += ni // 128
        s0 += ns
    return chunks, idx_col, out_col


def build_nc(hitsteps: frozenset):
    from contextlib import ExitStack

    chunks, idx_cols, out_blocks = _plan(S)
    NCH2 = 2 * len(chunks)

    nc = bacc.Bacc("TRN2", target_bir_lowering=False, debug=False, num_devices=8)

    ef_d = nc.dram_tensor("ef", [128, SF * 512], BF16, kind="ExternalInput").ap()
    eb_d = nc.dram_tensor("eb", [128, SF * 512], BF16, kind="ExternalInput").ap()
    tgv_d = nc.dram_tensor("tgv", [64, 256], F32, kind="ExternalInput").ap()
    mkf_d = nc.dram_tensor("mkf", [64, 256], F32, kind="ExternalInput").ap()
    s0b_d = nc.dram_tensor("s0b", [128, SF], F32, kind="ExternalInput").ap()
    injt_d = nc.dram_tensor("injt", [128, SF * 64], BF16, kind="ExternalInput").ap()
    m99_d = nc.dram_tensor("m99", [128, 16], F32, kind="ExternalInput").ap()
    end99_d = nc.dram_tensor("end99", [128, 16], F32, kind="ExternalInput").ap()
    cs_d = nc.dram_tensor("cs", [64, 1], F32, kind="ExternalInput").ap()
    lhsA_d = nc.dram_tensor("lhsA", [128, 64], BF16, kind="ExternalInput").ap()
    lhsI_d = nc.dram_tensor("lhsI", [128, 64], BF16, kind="ExternalInput").ap()
    lhsJ_d = nc.dram_tensor("lhsJ", [128, 64], F32, kind="ExternalInput").ap()
    out_d = nc.dram_tensor("losses", [64, 1], F32, kind="ExternalOutput").ap()

    with tile.TileContext(nc) as tc, ExitStack() as ctx:
        state = ctx.enter_context(tc.tile_pool(name="state", bufs=1))
        blkp = {
            d: ctx.enter_context(tc.tile_pool(name=f"blk{d}", bufs=3))
            for d in ("f", "b")
        }
        e16p = {
            d: ctx.enter_context(tc.tile_pool(name=f"e16{d}", bufs=3))
            for d in ("f", "b")
        }
        work = ctx.enter_context(tc.tile_pool(name="work", bufs=4))
        psf = ctx.enter_context(tc.tile_pool(name="psf", bufs=2, space="PSUM"))
        psb = ctx.enter_context(tc.tile_pool(name="psb", bufs=2, space="PSUM"))
        psumg = ctx.enter_context(tc.tile_pool(name="psumg", bufs=2, space="PSUM"))

        # ---- persistent tiles ----
        ones = state.tile([128, 1], F32)
        biast = state.tile([128, 1], F32)
        s0bt = state.tile([128, SF], F32)
        injt = state.tile([128, SF * 64], BF16)
        m99t = state.tile([128, 16], F32)
        end99t = state.tile([128, 16], F32)
        cst = state.tile([64, 1], F32)
        lhsA = state.tile([128, 64], BF16)
        lhsI = state.tile([128, 64], BF16)
        lhsJ = state.tile([128, 64], F32)
        scf = [state.tile([128, 512], BF16, name=f"scf{i}") for i in range(2)]
        scb = [state.tile([128, 512], BF16, name=f"scb{i}") for i in range(2)]
        tgv = state.tile([64, 256], F32)
        mkf = state.tile([64, 256], F32)

        # critical-path tables first (tiny), then the first data blocks in
        # halves (exp starts earlier); join-only tables are loaded late.
        nc.vector.memset(ones[:], 1.0)
        nc.vector.memset(biast[:], -CEXP)

        nc.sync.dma_start(lhsI[:], lhsI_d[:])
        nc.sync.dma_start(lhsA[:], lhsA_d[:])
        nc.sync.dma_start(injt[:, 0:64], injt_d[:, 0:64])
        nc.sync.dma_start(s0bt[:], s0b_d[:])

        # ---- streamed blocks + exp ----
        def load_block(d, bi, halves=1):
            src = ef_d if d == "f" else eb_d
            blk = blkp[d].tile([128, SBLK * 512], BF16, tag="blk", name=f"blkt{d}")
            c0 = bi * 2048
            hw_ = 2048 // halves
            for h in range(halves):
                nc.sync.dma_start(
                    blk[:, h * hw_ : (h + 1) * hw_],
                    src[:, c0 + h * hw_ : c0 + (h + 1) * hw_],
                )
            return blk

        def exp_block(d, blk, halves=1):
            e16 = e16p[d].tile([128, SBLK * 512], BF16, tag="e16", name=f"e16t{d}")
            hw_ = 2048 // halves
            for h in range(halves):
                nc.scalar.activation(
                    e16[:, h * hw_ : (h + 1) * hw_],
                    blk[:, h * hw_ : (h + 1) * hw_],
                    mybir.ActivationFunctionType.Exp,
                    bias=biast[:],
                )
            return e16

        blk = {d: load_block(d, 0, halves=4) for d in ("f", "b")}
        e16 = {d: exp_block(d, blk[d], halves=4) for d in ("f", "b")}
        blk_next = {d: load_block(d, 1) for d in ("f", "b")}
        e16_next = {d: exp_block(d, blk_next[d]) for d in ("f", "b")}
        blk_next2 = {d: load_block(d, 2) for d in ("f", "b")}
        e16_next2 = {d: exp_block(d, blk_next2[d]) for d in ("f", "b")}

        # join-only tables: needed only at the tail
        nc.sync.dma_start(injt[:, 64:], injt_d[:, 64:])
        nc.sync.dma_start(m99t[:], m99_d[:])
        nc.sync.dma_start(end99t[:], end99_d[:])
        nc.sync.dma_start(cst[:], cs_d[:])
        nc.sync.dma_start(lhsJ[:], lhsJ_d[:])
        nc.sync.dma_start(tgv[:], tgv_d[:])
        nc.sync.dma_start(mkf[:], mkf_d[:])

        # ---- fwd init: state[(h,b), j] = E_0[b, START=(1,14), (h,j)] ----
        ptf = psf.tile([128, 16], F32, tag="ptf")
        nc.tensor.matmul(
            ptf[0:64, :], lhsI[:], e16["f"][:, 14:256:16], start=True, stop=True
        )
        nc.tensor.matmul(
            ptf[64:128, :], lhsI[:], e16["f"][:, 270:512:16], start=True, stop=True
        )

        # ---- bwd init (k=199, jidx=0): inject-only ----
        ptb = psb.tile([128, 16], F32, tag="ptb")
        nc.tensor.matmul(
            ptb[0:64, :], injt[:, 0:64], e16["b"][:, 15:256:16], start=True, stop=True
        )
        nc.tensor.matmul(
            ptb[64:128, :], injt[:, 0:64], e16["b"][:, 271:512:16],
            start=True, stop=True,
        )

        # ---- main loop: wall step w handles fwd step 1+w, bwd jidx 1+w ----
        for w in range(NW):
            j = 1 + w                 # fwd step index == bwd stream index
            bi, sl = divmod(j, SBLK)
            if sl == 0:
                for d in ("f", "b"):
                    blk[d] = blk_next[d]
                    e16[d] = e16_next[d]
                blk_next = blk_next2
                e16_next = e16_next2
                if bi + 2 < NBLK:
                    blk_next2 = {d: load_block(d, bi + 2) for d in ("f", "b")}
                    e16_next2 = {d: exp_block(d, blk_next2[d]) for d in ("f", "b")}

            # fwd: scan -> 2 seg mms
            sc = scf[w % 2]
            nc.vector._custom_dve(
                MUL_SEGSCAN_SCALE,
                out=sc[:].rearrange("p (s n) -> p s n", n=16),
                in0=e16["f"][:, sl * 512 : (sl + 1) * 512].rearrange(
                    "p (s n) -> p s n", n=16
                ),
                in1=ptf[:].unsqueeze(1).broadcast_to([128, 32, 16]),
                s0=ones[:],
            )
            ptf = psf.tile([128, 16], F32, tag="ptf")
            nc.tensor.matmul(ptf[0:64, :], lhsA[:], sc[:, 15:256:16], start=True, stop=True)
            nc.tensor.matmul(ptf[64:128, :], lhsA[:], sc[:, 271:512:16], start=True, stop=True)

            # bwd: scan (s0 kill) -> 2 seg mms (+ inject mms on hit steps)
            sb_ = scb[w % 2]
            nc.vector._custom_dve(
                MUL_SEGSCAN_SCALE,
                out=sb_[:].rearrange("p (s n) -> p s n", n=16),
                in0=e16["b"][:, sl * 512 : (sl + 1) * 512].rearrange(
                    "p (s n) -> p s n", n=16
                ),
                in1=ptb[:].unsqueeze(1).broadcast_to([128, 32, 16]),
                s0=s0bt[:, j : j + 1],
            )
            ptb = psb.tile([128, 16], F32, tag="ptb")
            hit = j in hitsteps
            nc.tensor.matmul(
                ptb[0:64, :], lhsA[:], sb_[:, 15:256:16], start=True, stop=not hit
            )
            nc.tensor.matmul(
                ptb[64:128, :], lhsA[:], sb_[:, 271:512:16], start=True, stop=not hit
            )
            if hit:
                nc.tensor.matmul(
                    ptb[0:64, :], injt[:, j * 64 : j * 64 + 64],
                    e16["b"][:, sl * 512 + 15 : sl * 512 + 256 : 16],
                    start=False, stop=True,
                )
                nc.tensor.matmul(
                    ptb[64:128, :], injt[:, j * 64 : j * 64 + 64],
                    e16["b"][:, sl * 512 + 271 : sl * 512 + 512 : 16],
                    start=False, stop=True,
                )

        # ---- tg energy: masked sum of host-extracted target scores ----
        tgE = state.tile([64, 1], F32)
        tgtmp = work.tile([64, 256], F32, tag="tgtmp")
        nc.vector.scalar_tensor_tensor(
            tgtmp[:],
            tgv[:],
            1.0,
            mkf[:],
            op0=mybir.AluOpType.mult,
            op1=mybir.AluOpType.mult,
            accum_out=tgE[:],
        )

        # ---- join ----
        w2 = state.tile([128, 16], F32)
        nc.vector.tensor_mul(w2[:], ptb[:], m99t[:])
        nc.vector.tensor_add(w2[:], w2[:], end99t[:])
        prod = state.tile([128, 16], F32)
        nc.vector.tensor_mul(prod[:], w2[:], ptf[:])
        dsum = state.tile([128, 1], F32)
        nc.vector.reduce_sum(dsum[:], prod[:], axis=mybir.AxisListType.X)
        dps = psumg.tile([64, 1], F32, tag="d")
        nc.tensor.matmul(dps[:], lhsJ[:], dsum[:], start=True, stop=True)
        lnz = state.tile([64, 1], F32)
        nc.scalar.activation(lnz[:], dps[:], mybir.ActivationFunctionType.Ln)
        res = state.tile([64, 1], F32)
        nc.vector.tensor_add(res[:], lnz[:], cst[:])
        nc.vector.tensor_sub(res[:], res[:], tgE[:])
        nc.sync.dma_start(out_d[:], res[:])

    nc.compile()
    return nc


def host_prep(scores_a: np.ndarray, targets_a: np.ndarray, mask: np.ndarray):
    """Per-annotator tensors for the v2 kernel."""
    chunks, idx_cols, out_blocks = _plan(S)

    lens = mask.astype(np.int64).sum(axis=0)  # (B,)
    assert lens.min() >= S // 2, "kernel assumes valid-prefix lens >= S//2"
    sbv = lens - 1  # cutoff step per b in [99, 199]

    x = scores_a.reshape(S, B, 2, 16, 2, 16)  # s b h j th tl
    arr_f = np.ascontiguousarray(
        x[:SF].transpose(2, 1, 0, 4, 5, 3)       # h b s th tl j
    ).astype(ml_dtypes.bfloat16).reshape(128, SF * 512)
    # bwd: rows (tt,b); col (jidx, hf, fl, tl); jidx -> k = 199 - jidx
    xb = x[SF:][::-1]                             # jidx b hf fl tt tl
    arr_b = np.ascontiguousarray(
        xb.transpose(4, 1, 0, 2, 3, 5)            # tt b jidx hf fl tl
    ).astype(ml_dtypes.bfloat16).reshape(128, SF * 512)

    # s0 kill + inject tables (rows (x2, b64))
    r = np.arange(128)
    br = r % 64
    s0b = np.ones((128, SF), dtype=np.float32)
    injt = np.zeros((128, SF, 64), dtype=np.float32)
    lhsI_base = ((br[:, None] == np.arange(64)[None, :]) & (r[:, None] >= 64))
    for jidx in range(SF):
        k = S - 1 - jidx
        hit = sbv == k                            # (B,)
        s0b[:, jidx] = (~hit)[br]
        injt[:, jidx, :] = lhsI_base * hit[None, :]
    injt = injt.reshape(128, SF * 64).astype(ml_dtypes.bfloat16)

    m99 = np.repeat((~(sbv == SF - 1))[br].astype(np.float32)[:, None], 16, axis=1)
    end99 = np.zeros((128, 16), dtype=np.float32)
    for b in range(B):
        if sbv[b] == SF - 1:
            end99[64 + b, 15] = 1.0
    cs = (CEXP * (sbv + 1)).astype(np.float32)[:, None]

    lhsAf = (br[:, None] == np.arange(64)[None, :]).astype(np.float32)
    lhsA = lhsAf.astype(ml_dtypes.bfloat16)
    lhsI = lhsI_base.astype(ml_dtypes.bfloat16)
    lhsJ = lhsAf.copy()

    # tg values: host-side indexed extraction (pure data movement);
    # the mask multiply + sum stay on device.
    tgt = targets_a.astype(np.int64)              # (S, B)
    flat = scores_a.reshape(S, B, T * T)
    tgvals = np.take_along_axis(flat, tgt[..., None], axis=2)[..., 0]  # (S, B)
    tgv = np.zeros((64, 256), dtype=np.float32)
    tgv[:, :S] = tgvals.T
    mkf = np.zeros((64, 256), dtype=np.float32)
    mkf[:, :S] = mask.T.astype(np.float32)

    return dict(
        ef=arr_f, eb=arr_b, tgv=tgv, mkf=mkf,
        s0b=s0b, injt=injt, m99=m99, end99=end99, cs=cs,
        lhsA=lhsA, lhsI=lhsI, lhsJ=lhsJ,
    )


_NC_CACHE = {}

TRACE = False
TRACE_DIR = None
LAST_RESULTS = None


def _get_nc(hitsteps: frozenset):
    if hitsteps not in _NC_CACHE:
        _NC_CACHE[hitsteps] = build_nc(hitsteps)
    return _NC_CACHE[hitsteps]


def kernel(scores, targets, mask, a_mask):
    scores = np.asarray(scores)
    targets = np.asarray(targets)
    mask_np = np.asarray(mask).astype(bool)
    a_mask_np = np.asarray(a_mask).astype(bool)

    lens = mask_np.astype(np.int64).sum(axis=0)
    sbv = lens - 1
    hitsteps = frozenset(int(S - 1 - k) for k in sbv if k >= SF) - {0}
    nc = _get_nc(hitsteps)

    in_maps = []
    for a in range(A):
        in_maps.append(host_prep(scores[a], targets[a], mask_np))

    if TRACE:
        import antenv

        shim = "/opt/trn_rl_repo/antenv"
        if os.path.isdir(shim) and shim not in list(antenv.__path__):
            antenv.__path__.append(shim)

    global LAST_RESULTS
    res = run_bass_kernel_spmd(
        nc, in_maps, core_ids=list(range(A)), trace=TRACE, tmpdir=TRACE_DIR
    )
    LAST_RESULTS = res
    losses = np.stack([r["losses"][:, 0] for r in res.results])  # (A, B)
    loss = np.where(a_mask_np, losses, 0.0).sum(dtype=np.float32) / np.float32(B)
    return np.float32(loss)


# revision 16
# speedup vs baseline: 1.0170x; 1.0043x over previous
"""CRF loss (multi-annotator) Trainium2 kernel.

Problem (hardcoded): scores (8,200,64,32,32) f32, targets (8,200,64) int,
mask (200,64) bool, a_mask (8,64) bool -> scalar f32 loss.

Sharding: one annotator per NeuronCore (8 cores). Host applies a_mask and
sums / B.

Design:
  - The sequence is split into two independent serial chains: FORWARD over
    steps 1..99 (mask-free: all lens >= 100) and BACKWARD over steps
    199..100.  log_Z[b] = ln <p_fwd99[b], w_bwd100[b]> + C*(sb+1); the
    backward chain computes w_k = M_k ... M_sb 1_END by injecting
    E_k[:, END] at each batch's cutoff step (per-partition s0 kill in the
    scan + inject-matmuls whose lhsT carries the host-baked hit mask).
    Two chains halve the serial length and hide the scan->mm->scan latency.
  - Layout rows=(half2, b64): all 64 batch elements on partitions.  One
    [128,512] fused custom DVE op per direction per step: a SEGMENTED
    multiply-scan (product e*state*s0, running sum reset every 16
    elements via a hand-built SUB_DIM_DONE step-state in the uop FSM).
    Segment sums are then direct samples at columns 15::16, so the state
    update is just 2 bf16 matmuls (batch-dup lhsT; rows 0:64/64:128 of
    one PSUM tile) and the next scan reads the state straight from PSUM.
  - No renormalization: the exp bias -ln(52.76) is fused into the ACT
    activation (bias operand), keeping the un-normalized state within f32
    range over 99 steps; a single Ln at the join recovers the log.
  - exp on ACT in [128,2048] blocks, double-buffered two blocks deep with
    the DMA stream; fwd stream carries steps 0..99, bwd stream 199..100,
    so each score element is streamed exactly once (26 MB bf16 per core).
  - tg energy: target score values are extracted host-side (pure indexed
    data movement, like the layout transforms); the mask multiply and
    reduction stay on device (one STT with accumulate).
"""

import os
import sys

import numpy as np

if os.path.isdir("/opt/trn_rl_repo"):
    sys.path.insert(0, "/opt/trn_rl_repo")

import ml_dtypes  # noqa: E402

import concourse.bass as bass  # noqa: E402
import concourse.tile as tile  # noqa: E402
from concourse import bacc, mybir  # noqa: E402
from concourse.bass_utils import run_bass_kernel_spmd  # noqa: E402

F32 = mybir.dt.float32
BF16 = mybir.dt.bfloat16
I16 = mybir.dt.int16

A, S, B, T = 8, 200, 64, 32
START_TAG, END_TAG = 30, 31
SF = 100        # fwd: steps 0..SF-1 (scan 1..SF-1); bwd: steps S-1..SF
NW = SF - 1     # wall steps in the main loop (99)
SBLK = 4        # steps per streamed DMA block
NBLK = SF // SBLK  # 25 blocks per direction
GBLK = 16       # steps per dma_gather chunk
CEXP = 3.9656   # exp bias: E = exp(x - CEXP)

# ---------------------------------------------------------------------------
# Custom DVE op: out[k] = running_sum(in0*in1*s0) (inclusive, whole stream)
# ---------------------------------------------------------------------------


def _register_mul_segscan():
    """out[p, s, k] = cumsum_k(in0[p,s,k] * in1[p,s,k] * s0[p]) with the
    running sum RESET at each page boundary (segmented scan over the
    innermost dim of a [P, S, N] access pattern).

    Built from the stock lower() pieces: same placement as the plain
    mul-cumsum scan, plus the SUB_DIM_DONE -> step-state transition (the
    proven PageIdx FSM shape) whose single-element override re-seeds the
    accumulator with the incoming product (BYPASS instead of ADD).
    """
    import dataclasses

    import concourse.dve_ops as dve_ops
    from concourse.dve_ops import OPS, DveOp, DveOpSpec
    from concourse import dve_spec as ds
    from concourse.dve_spec import AluOp, Spec, Src0, Src1, C0, scan
    from concourse.dve_uop import AluInp, Trigger

    name = "MUL_SEGSCAN_SCALE"
    for op in OPS:
        if op.name == name:
            return op

    def _ref(in0, in1, s0):
        prod = in0.astype(np.float32) * in1 * s0
        return np.cumsum(prod, axis=-1)

    spec = Spec(body=scan(AluOp.ADD, Src0 * Src1 * C0), reference=_ref)

    def lower_seg(spec, ver):
        from concourse.dve_uop import N_LANES, N_STAGES

        n_lanes, n_stages = N_LANES[ver], N_STAGES[ver]
        ds._validate_body(spec, ver)
        spec2 = ds._hoist_stream_invariant_ops(spec)
        scans = ds._collect(spec2.body, ds.Scan)
        latches = ds._collect(spec2.body, ds.Latch)
        placement = ds._build_placement(spec2, scans, n_stages, n_lanes)
        states = ds._build_state_machine(spec2, scans, latches, placement)
        assert len(states) == 2, states  # [seed, steady]
        steady = states[1]
        # steady: also fire step at each sub-dim boundary
        states[1] = dataclasses.replace(
            steady,
            trigger=(Trigger.SRC_TENSOR_DONE, Trigger.SUB_DIM_DONE, Trigger.NONE),
            next=(0, 2, 0),
        )
        # step: one element with the scan stage re-seeded from the product
        (sc_node,) = scans
        d = placement.node_stage[sc_node]
        states.append(
            dataclasses.replace(
                steady,
                overrides={d: ds._Stage(AluOp.BYPASS, AluInp.PREV_ALU_OUT)},
                trigger=(Trigger.SRC_TENSOR_DONE, Trigger.SUB_DIM_DONE, Trigger.COUNT),
                next=(0, 2, 1),
                repeat=1,
            )
        )
        out = [ds._assemble(st) for st in states]
        for u in out:
            u.validate(ver)
        return out

    row = dve_ops._CUSTOM_DVE_ROW_BASE + len(OPS)
    shas = {}
    for ver in ("v3", "v4"):
        shas[ver] = DveOpSpec(
            name=name, opcode=row, uops=lower_seg(spec, ver), rd1_en=True
        ).sha(ver)
    op = DveOp(name, spec, subdim=True, uops_sha=shas)
    # DveOp.__post_init__/compile paths re-lower via the stock lower();
    # patch this op's compiled entry into the cache so our uops are used.
    for ver in ("v3", "v4"):
        dve_ops._COMPILE_CACHE[(name, ver)] = DveOpSpec(
            name=name, opcode=row, uops=lower_seg(spec, ver), rd1_en=True
        )
    OPS.append(op)
    dve_ops.CUSTOM_DVE_SPECS[name] = spec
    dve_ops._SUB_OPCODE_FOR_NAME[name] = row
    return op


MUL_SEGSCAN_SCALE = _register_mul_segscan()


def _plan(S):
    """Gather chunk plan: list of (s0, nsteps, idx_col0, out_col0)."""
    chunks = []
    s0 = 0
    idx_col = 0
    out_col = 0
    while s0 < S:
        ns = min(GBLK, S - s0)
        ni = ns * B
        assert ni % 128 == 0
        chunks.append((s0, ns, idx_col, out_col))
        idx_col += ni // 16
        out_col += ni // 128
        s0 += ns
    return chunks, idx_col, out_col


def build_nc(hitsteps: frozenset):
    from contextlib import ExitStack

    chunks, idx_cols, out_blocks = _plan(S)
    NCH2 = 2 * len(chunks)

    nc = bacc.Bacc("TRN2", target_bir_lowering=False, debug=False, num_devices=8)

    ef_d = nc.dram_tensor("ef", [128, SF * 512], BF16, kind="ExternalInput").ap()
    eb_d = nc.dram_tensor("eb", [128, SF * 512], BF16, kind="ExternalInput").ap()
    tgv_d = nc.dram_tensor("tgv", [64, 256], F32, kind="ExternalInput").ap()
    mkf_d = nc.dram_tensor("mkf", [64, 256], F32, kind="ExternalInput").ap()
    s0b_d = nc.dram_tensor("s0b", [128, SF], F32, kind="ExternalInput").ap()
    injt_d = nc.dram_tensor("injt", [128, SF * 64], BF16, kind="ExternalInput").ap()
    m99_d = nc.dram_tensor("m99", [128, 16], F32, kind="ExternalInput").ap()
    end99_d = nc.dram_tensor("end99", [128, 16], F32, kind="ExternalInput").ap()
    cs_d = nc.dram_tensor("cs", [64, 1], F32, kind="ExternalInput").ap()
    lhsA_d = nc.dram_tensor("lhsA", [128, 64], BF16, kind="ExternalInput").ap()
    lhsI_d = nc.dram_tensor("lhsI", [128, 64], BF16, kind="ExternalInput").ap()
    lhsJ_d = nc.dram_tensor("lhsJ", [128, 64], F32, kind="ExternalInput").ap()
    out_d = nc.dram_tensor("losses", [64, 1], F32, kind="ExternalOutput").ap()

    with tile.TileContext(nc) as tc, ExitStack() as ctx:
        state = ctx.enter_context(tc.tile_pool(name="state", bufs=1))
        blkp = {
            d: ctx.enter_context(tc.tile_pool(name=f"blk{d}", bufs=3))
            for d in ("f", "b")
        }
        e16p = {
            d: ctx.enter_context(tc.tile_pool(name=f"e16{d}", bufs=3))
            for d in ("f", "b")
        }
        work = ctx.enter_context(tc.tile_pool(name="work", bufs=4))
        psf = ctx.enter_context(tc.tile_pool(name="psf", bufs=2, space="PSUM"))
        psb = ctx.enter_context(tc.tile_pool(name="psb", bufs=2, space="PSUM"))
        psumg = ctx.enter_context(tc.tile_pool(name="psumg", bufs=2, space="PSUM"))

        # ---- persistent tiles ----
        ones = state.tile([128, 1], F32)
        biast = state.tile([128, 1], F32)
        s0bt = state.tile([128, SF], F32)
        injt = state.tile([128, SF * 64], BF16)
        m99t = state.tile([128, 16], F32)
        end99t = state.tile([128, 16], F32)
        cst = state.tile([64, 1], F32)
        lhsA = state.tile([128, 64], BF16)
        lhsI = state.tile([128, 64], BF16)
        lhsJ = state.tile([128, 64], F32)
        scf = [state.tile([128, 512], BF16, name=f"scf{i}") for i in range(2)]
        scb = [state.tile([128, 512], BF16, name=f"scb{i}") for i in range(2)]
        tgv = state.tile([64, 256], F32)
        mkf = state.tile([64, 256], F32)

        # critical-path tables first (tiny), then the first data blocks in
        # halves (exp starts earlier); join-only tables are loaded late.
        nc.vector.memset(ones[:], 1.0)
        nc.vector.memset(biast[:], -CEXP)

        nc.sync.dma_start(lhsI[:], lhsI_d[:])
        nc.sync.dma_start(lhsA[:], lhsA_d[:])
        nc.sync.dma_start(injt[:, 0:64], injt_d[:, 0:64])
        nc.sync.dma_start(s0bt[:], s0b_d[:])

        # ---- streamed blocks + exp ----
        def load_block(d, bi, halves=1):
            src = ef_d if d == "f" else eb_d
            blk = blkp[d].tile([128, SBLK * 512], BF16, tag="blk", name=f"blkt{d}")
            c0 = bi * 2048
            hw_ = 2048 // halves
            for h in range(halves):
                nc.sync.dma_start(
                    blk[:, h * hw_ : (h + 1) * hw_],
                    src[:, c0 + h * hw_ : c0 + (h + 1) * hw_],
                )
            return blk

        def exp_block(d, blk, halves=1):
            e16 = e16p[d].tile([128, SBLK * 512], BF16, tag="e16", name=f"e16t{d}")
            hw_ = 2048 // halves
            for h in range(halves):
                nc.scalar.activation(
                    e16[:, h * hw_ : (h + 1) * hw_],
                    blk[:, h * hw_ : (h + 1) * hw_],
                    mybir.ActivationFunctionType.Exp,
                    bias=biast[:],
                )
            return e16

        blk = {d: load_block(d, 0, halves=4) for d in ("f", "b")}
        e16 = {d: exp_block(d, blk[d], halves=4) for d in ("f", "b")}
        blk_next = {d: load_block(d, 1) for d in ("f", "b")}
        e16_next = {d: exp_block(d, blk_next[d]) for d in ("f", "b")}
        blk_next2 = {d: load_block(d, 2) for d in ("f", "b")}
        e16_next2 = {d: exp_block(d, blk_next2[d]) for d in ("f", "b")}

        # join-only tables: needed only at the tail
        nc.sync.dma_start(injt[:, 64:], injt_d[:, 64:])
        nc.sync.dma_start(m99t[:], m99_d[:])
        nc.sync.dma_start(end99t[:], end99_d[:])
        nc.sync.dma_start(cst[:], cs_d[:])
        nc.sync.dma_start(lhsJ[:], lhsJ_d[:])
        nc.sync.dma_start(tgv[:], tgv_d[:])
        nc.sync.dma_start(mkf[:], mkf_d[:])

        # ---- fwd init: state[(h,b), j] = E_0[b, START=(1,14), (h,j)] ----
        ptf = psf.tile([128, 16], F32, tag="ptf")
        nc.tensor.matmul(
            ptf[0:64, :], lhsI[:], e16["f"][:, 14:256:16], start=True, stop=True
        )
        nc.tensor.matmul(
            ptf[64:128, :], lhsI[:], e16["f"][:, 270:512:16], start=True, stop=True
        )

        # ---- bwd init (k=199, jidx=0): inject-only ----
        ptb = psb.tile([128, 16], F32, tag="ptb")
        nc.tensor.matmul(
            ptb[0:64, :], injt[:, 0:64], e16["b"][:, 15:256:16], start=True, stop=True
        )
        nc.tensor.matmul(
            ptb[64:128, :], injt[:, 0:64], e16["b"][:, 271:512:16],
            start=True, stop=True,
        )

        # ---- main loop: wall step w handles fwd step 1+w, bwd jidx 1+w ----
        for w in range(NW):
            j = 1 + w                 # fwd step index == bwd stream index
            bi, sl = divmod(j, SBLK)
            if sl == 0:
                for d in ("f", "b"):
                    blk[d] = blk_next[d]
                    e16[d] = e16_next[d]
                blk_next = blk_next2
                e16_next = e16_next2
                if bi + 2 < NBLK:
                    blk_next2 = {d: load_block(d, bi + 2) for d in ("f", "b")}
                    e16_next2 = {d: exp_block(d, blk_next2[d]) for d in ("f", "b")}

            # fwd: scan -> 2 seg mms
            sc = scf[w % 2]
            nc.vector._custom_dve(
                MUL_SEGSCAN_SCALE,
                out=sc[:].rearrange("p (s n) -> p s n", n=16),
                in0=e16["f"][:, sl * 512 : (sl + 1) * 512].rearrange(
                    "p (s n) -> p s n", n=16
                ),
                in1=ptf[:].unsqueeze(1).broadcast_to([128, 32, 16]),
                s0=ones[:],
            )
            ptf = psf.tile([128, 16], F32, tag="ptf")
            nc.tensor.matmul(ptf[0:64, :], lhsA[:], sc[:, 15:256:16], start=True, stop=True)
            nc.tensor.matmul(ptf[64:128, :], lhsA[:], sc[:, 271:512:16], start=True, stop=True)

            # bwd: scan (s0 kill) -> 2 seg mms (+ inject mms on hit steps)
            sb_ = scb[w % 2]
            nc.vector._custom_dve(
                MUL_SEGSCAN_SCALE,
                out=sb_[:].rearrange("p (s n) -> p s n", n=16),
                in0=e16["b"][:, sl * 512 : (sl + 1) * 512].rearrange(
                    "p (s n) -> p s n", n=16
                ),
                in1=ptb[:].unsqueeze(1).broadcast_to([128, 32, 16]),
                s0=s0bt[:, j : j + 1],
            )
            ptb = psb.tile([128, 16], F32, tag="ptb")
            hit = j in hitsteps
            nc.tensor.matmul(
                ptb[0:64, :], lhsA[:], sb_[:, 15:256:16], start=True, stop=not hit
            )
            nc.tensor.matmul(
                ptb[64:128, :], lhsA[:], sb_[:, 271:512:16], start=True, stop=not hit
            )
            if hit:
                nc.tensor.matmul(
                    ptb[0:64, :], injt[:, j * 64 : j * 64 + 64],
                    e16["b"][:, sl * 512 + 15 : sl * 512 + 256 : 16],
                    start=False, stop=True,
                )
                nc.tensor.matmul(
                    ptb[64:128, :], injt[:, j * 64 : j * 64 + 64],
                    e16["b"][:, sl * 512 + 271 : sl * 512 + 512 : 16],
                    start=False, stop=True,
                )

        # ---- tg energy: masked sum of host-extracted target scores ----
        tgE = state.tile([64, 1], F32)
        tgtmp = work.tile([64, 256], F32, tag="tgtmp")
        nc.vector.scalar_tensor_tensor(
            tgtmp[:],
            tgv[:],
            1.0,
            mkf[:],
            op0=mybir.AluOpType.mult,
            op1=mybir.AluOpType.mult,
            accum_out=tgE[:],
        )

        # ---- join ----
        w2 = state.tile([128, 16], F32)
        nc.vector.tensor_mul(w2[:], ptb[:], m99t[:])
        nc.vector.tensor_add(w2[:], w2[:], end99t[:])
        prod = state.tile([128, 16], F32)
        nc.vector.tensor_mul(prod[:], w2[:], ptf[:])
        dsum = state.tile([128, 1], F32)
        nc.vector.reduce_sum(dsum[:], prod[:], axis=mybir.AxisListType.X)
        dps = psumg.tile([64, 1], F32, tag="d")
        nc.tensor.matmul(dps[:], lhsJ[:], dsum[:], start=True, stop=True)
        lnz = state.tile([64, 1], F32)
        nc.scalar.activation(lnz[:], dps[:], mybir.ActivationFunctionType.Ln)
        res = state.tile([64, 1], F32)
        nc.vector.tensor_add(res[:], lnz[:], cst[:])
        nc.vector.tensor_sub(res[:], res[:], tgE[:])
        nc.sync.dma_start(out_d[:], res[:])

    nc.compile()
    return nc


def host_prep(scores_a: np.ndarray, targets_a: np.ndarray, mask: np.ndarray):
    """Per-annotator tensors for the v2 kernel."""
    chunks, idx_cols, out_blocks = _plan(S)

    lens = mask.astype(np.int64).sum(axis=0)  # (B,)
    assert lens.min() >= S // 2, "kernel assumes valid-prefix lens >= S//2"
    sbv = lens - 1  # cutoff step per b in [99, 199]

    x = scores_a.reshape(S, B, 2, 16, 2, 16)  # s b h j th tl
    arr_f = np.ascontiguousarray(
        x[:SF].transpose(2, 1, 0, 4, 5, 3)       # h b s th tl j
    ).astype(ml_dtypes.bfloat16).reshape(128, SF * 512)
    # bwd: rows (tt,b); col (jidx, hf, fl, tl); jidx -> k = 199 - jidx
    xb = x[SF:][::-1]                             # jidx b hf fl tt tl
    arr_b = np.ascontiguousarray(
        xb.transpose(4, 1, 0, 2, 3, 5)            # tt b jidx hf fl tl
    ).astype(ml_dtypes.bfloat16).reshape(128, SF * 512)

    # s0 kill + inject tables (rows (x2, b64))
    r = np.arange(128)
    br = r % 64
    s0b = np.ones((128, SF), dtype=np.float32)
    injt = np.zeros((128, SF, 64), dtype=np.float32)
    lhsI_base = ((br[:, None] == np.arange(64)[None, :]) & (r[:, None] >= 64))
    for jidx in range(SF):
        k = S - 1 - jidx
        hit = sbv == k                            # (B,)
        s0b[:, jidx] = (~hit)[br]
        injt[:, jidx, :] = lhsI_base * hit[None, :]
    injt = injt.reshape(128, SF * 64).astype(ml_dtypes.bfloat16)

    m99 = np.repeat((~(sbv == SF - 1))[br].astype(np.float32)[:, None], 16, axis=1)
    end99 = np.zeros((128, 16), dtype=np.float32)
    for b in range(B):
        if sbv[b] == SF - 1:
            end99[64 + b, 15] = 1.0
    cs = (CEXP * (sbv + 1)).astype(np.float32)[:, None]

    lhsAf = (br[:, None] == np.arange(64)[None, :]).astype(np.float32)
    lhsA = lhsAf.astype(ml_dtypes.bfloat16)
    lhsI = lhsI_base.astype(ml_dtypes.bfloat16)
    lhsJ = lhsAf.copy()

    # tg values: host-side indexed extraction (pure data movement);
    # the mask multiply + sum stay on device.
    tgt = targets_a.astype(np.int64)              # (S, B)
    flat = scores_a.reshape(S, B, T * T)
    tgvals = np.take_along_axis(flat, tgt[..., None], axis=2)[..., 0]  # (S, B)
    tgv = np.zeros((64, 256), dtype=np.float32)
    tgv[:, :S] = tgvals.T
    mkf = np.zeros((64, 256), dtype=np.float32)
    mkf[:, :S] = mask.T.astype(np.float32)

    return dict(
        ef=arr_f, eb=arr_b, tgv=tgv, mkf=mkf,
        s0b=s0b, injt=injt, m99=m99, end99=end99, cs=cs,
        lhsA=lhsA, lhsI=lhsI, lhsJ=lhsJ,
    )


_NC_CACHE = {}

TRACE = False
TRACE_DIR = None
LAST_RESULTS = None


def _get_nc(hitsteps: frozenset):
    if hitsteps not in _NC_CACHE:
        _NC_CACHE[hitsteps] = build_nc(hitsteps)
    return _NC_CACHE[hitsteps]


def kernel(scores, targets, mask, a_mask):
    scores = np.asarray(scores)
    targets = np.asarray(targets)
    mask_np = np.asarray(mask).astype(bool)
    a_mask_np = np.asarray(a_mask).astype(bool)

    lens = mask_np.astype(np.int64).sum(axis=0)
    sbv = lens - 1
    hitsteps = frozenset(int(S - 1 - k) for k in sbv if k >= SF) - {0}
    nc = _get_nc(hitsteps)

    in_maps = []
    for a in range(A):
        in_maps.append(host_prep(scores[a], targets[a], mask_np))

    if TRACE:
        import antenv

        shim = "/opt/trn_rl_repo/antenv"
        if os.path.isdir(shim) and shim not in list(antenv.__path__):
            antenv.__path__.append(shim)

    global LAST_RESULTS
    res = run_bass_kernel_spmd(
        nc, in_maps, core_ids=list(range(A)), trace=TRACE, tmpdir=TRACE_DIR
    )
    LAST_RESULTS = res
    losses = np.stack([r["losses"][:, 0] for r in res.results])  # (A, B)
    loss = np.where(a_mask_np, losses, 0.0).sum(dtype=np.float32) / np.float32(B)
    return np.float32(loss)
